# revision 113
# baseline (speedup 1.0000x reference)
"""ContrastiveLoss kernel for Trainium2 (8 NeuronCores, Bass/Tile).

loss = sum(sgn * dist) - sum(sgn * sim)  over all N^2 pairs, where
  sgn = +1 if y_i == y_j else -1
  dist = sqrt(relu(sq_i + sq_j - 2 * gram_ij))
  sim  = gram_ij / (norm_i * norm_j)

Device strategy (row-sharded across 8 cores, 512 rows each):
  - PE: PSUM = gram_block - (sq_i + sq_j)/2  (rank-2 seed matmul + bf16 gram)
  - ACT: r = Relu(-2 * PSUM) = relu(d2);  dist = Sqrt(r)
  - DVE: scalar_tensor_tensor (dist * hsgn) with accum_out row-sums,
         hsgn = (label_j == label_i) - 0.5 in {+0.5, -0.5}
  - sim term via class sums: sum(sgn*sim) = 2*sum_c ||S_c||^2 - ||S||^2,
    S_c = sum_{y_i=c} f_i/||f_i||  -> tiny one-hot matmul, partials per core.
Host: sums per-core partials (fp64) and combines.
"""

import numpy as np
import ml_dtypes
from contextlib import ExitStack

N = 4096
D = 1024
NCORES = 8
CHUNK = N // NCORES          # 512 rows per core
RT = CHUNK // 128            # 4 row tiles per core
CT = N // 512                # 8 column tiles
KT = D // 128                # 8 k tiles
NCLS = 100
COS_EPS = 1e-8
USE_FP8 = True  # fp8e4 DoubleRow gram (~1.4x PE) vs bf16
DIAG_C = 512.0  # diagonal d2 shift: d2_ii += 2*C so sqrt needs no relu

_BASS_CACHE = {}
LAST_RESULTS = None
_LAST_IN_MAPS = None


def _build_bass():
    import concourse.bass as bass
    import concourse.bacc as bacc
    import concourse.tile as tile
    from concourse import mybir

    dt = mybir.dt
    Alu = mybir.AluOpType
    Act = mybir.ActivationFunctionType

    nc = bacc.Bacc("TRN2", debug=False, num_devices=NCORES)

    fdt = dt.float8e4 if USE_FP8 else dt.bfloat16
    ftT = nc.dram_tensor("ftT", [D, N], fdt, kind="ExternalInput").ap()
    ftTc = nc.dram_tensor("ftTc", [D, CHUNK], fdt, kind="ExternalInput").ap()
    fc = nc.dram_tensor("fc", [CHUNK, D], fdt, kind="ExternalInput").ap()
    # DoubleRow seed operands (fp8, hi+lo split keeps bf16-level precision):
    # out[m,n] = c_hi_i[m] + c_lo_i[m] + c_hi_j[n] + c_lo_j[n]
    seeddrL = nc.dram_tensor("seeddrL", [2, 2, CHUNK], fdt, kind="ExternalInput").ap()
    seeddrR = nc.dram_tensor("seeddrR", [2, 2, N], fdt, kind="ExternalInput").ap()
    # sgn matmul operands (fp8 DoubleRow, Ko=1 rows zero):
    # lhsT rows 0..99 = 2*onehot chunk, row 100 = ones;
    # rhs  rows 0..99 = onehot,        row 100 = -ones
    # -> psum = 2*eq - 1 = sgn exactly
    ohTdrc = nc.dram_tensor("ohTdrc", [128, 2, CHUNK], fdt, kind="ExternalInput").ap()
    # rhs side stored without the zero Ko=1 plane; broadcast via step-0 AP
    ohTdr = nc.dram_tensor("ohTdr", [128, N], fdt, kind="ExternalInput").ap()
    rnc = nc.dram_tensor("rnc", [128, RT], dt.float32, kind="ExternalInput").ap()
    oh = nc.dram_tensor("oh", [CHUNK, NCLS + 1], fdt, kind="ExternalInput").ap()
    # diag shift (rows interleaved c::8), fp8 DR: (-8*I) x (64*dsel) = -512*diag
    negci = nc.dram_tensor("negci", [128, 2, 128], fdt, kind="ExternalInput").ap()
    dsel = nc.dram_tensor("dsel", [128, 1024], fdt, kind="ExternalInput").ap()

    rowacc = nc.dram_tensor(
        "rowacc", [128, RT * CT], dt.float32, kind="ExternalOutput"
    ).ap()
    scpart = nc.dram_tensor(
        "scpart", [NCLS + 1, D], dt.bfloat16, kind="ExternalOutput"
    ).ap()

    with tile.TileContext(nc) as tc, ExitStack() as ctx:
        singles = ctx.enter_context(tc.tile_pool(name="singles", bufs=1))
        psums = ctx.enter_context(tc.tile_pool(name="psums", bufs=4, space="PSUM"))
        eqps = ctx.enter_context(tc.tile_pool(name="eqps", bufs=3, space="PSUM"))
        psum_sc = ctx.enter_context(tc.tile_pool(name="psum_sc", bufs=1, space="PSUM"))
        work = ctx.enter_context(tc.tile_pool(name="work", bufs=4))

        # ---- loads ordered so the first gram tiles' data lands first -----
        seedL_sb = singles.tile([2, 2, CHUNK], fdt)
        nc.sync.dma_start(seedL_sb, seeddrL)
        seedR_sb = singles.tile([2, 2, N], fdt)
        nc.sync.dma_start(seedR_sb, seeddrR)

        # consolidated 3D DMAs (Bacc legalizes multi-sem consumer waits)
        ftTc_sb = singles.tile([128, KT, CHUNK], fdt)
        for h in range(2):  # halves so the first tiles can start sooner
            nc.sync.dma_start(
                ftTc_sb[:, 4 * h : 4 * h + 4, :],
                ftTc[512 * h : 512 * h + 512, :].rearrange(
                    "(kt p) n -> p kt n", p=128
                ),
            )
        ftT_sb = singles.tile([128, KT, N], fdt)

        def load_ft_col(j, split=False):
            for h in range(2 if split else 1):
                kts = slice(4 * h, 4 * h + 4) if split else slice(0, KT)
                rows = slice(512 * h, 512 * h + 512) if split else slice(0, D)
                nc.sync.dma_start(
                    ftT_sb[:, kts, j * 512 : (j + 1) * 512],
                    ftT[rows, j * 512 : (j + 1) * 512].rearrange(
                        "(kt p) n -> p kt n", p=128
                    ),
                )

        load_ft_col(0, split=True)

        negci_sb = singles.tile([128, 2, 128], fdt)
        nc.sync.dma_start(negci_sb, negci)
        dsel_sb = singles.tile([128, 1024], fdt)
        nc.sync.dma_start(dsel_sb, dsel)

        ohT_sb = singles.tile([128, N], fdt)
        nc.sync.dma_start(ohT_sb, ohTdr)
        ohTc_sb = singles.tile([128, 2, CHUNK], fdt)
        nc.sync.dma_start(ohTc_sb, ohTdrc)

        def bcast2(apx):
            # insert a broadcast (step 0, count 2) middle dim for DR rhs
            return bass.AP(
                tensor=apx.tensor,
                offset=apx.offset,
                ap=[list(apx.ap[0]), [0, 2], list(apx.ap[1])],
            )
        for j in range(1, CT):
            load_ft_col(j)

        # B-term inputs load after the gram stream (consumed mid-kernel)
        rnc_sb = singles.tile([128, RT], dt.float32)
        nc.sync.dma_start(rnc_sb, rnc)
        fc_sb = singles.tile([128, RT, D], fdt)
        nc.sync.dma_start(fc_sb, fc.rearrange("(rt p) d -> p rt d", p=128))
        oh_sb = singles.tile([128, RT, NCLS + 1], fdt)
        nc.sync.dma_start(oh_sb, oh.rearrange("(rt p) c -> p rt c", p=128))

        acc_sb = singles.tile([128, RT * CT], dt.float32)
        bias2048 = singles.tile([128, 1], dt.float32)
        nc.vector.memset(bias2048, 2048.0)

        # B term mid-kernel: u = fc * rnorm (gpsimd), per-class sums (PE).
        # Two sequential 1-bank PSUM groups keep the bank budget at 8.
        u_sb = singles.tile([128, RT, D], dt.bfloat16)
        for t in range(RT):
            nc.gpsimd.tensor_scalar(
                u_sb[:, t], fc_sb[:, t], rnc_sb[:, t : t + 1], None, Alu.mult
            )
        sc_sb = singles.tile([NCLS + 1, D], dt.bfloat16)
        for nh in range(2):
            scp = psum_sc.tile([NCLS + 1, 512], dt.float32, tag="sc")
            for t in range(RT):
                nc.tensor.matmul(
                    scp,
                    oh_sb[:, t, :],
                    u_sb[:, t, nh * 512 : (nh + 1) * 512],
                    start=(t == 0),
                    stop=(t == RT - 1),
                )
            nc.vector.tensor_copy(sc_sb[:, nh * 512 : (nh + 1) * 512], scp)
        nc.sync.dma_start(scpart, sc_sb)

        # ---- main loop: j outer so each 0.5MB ftT column block is used by
        # all 4 row tiles before the next block is needed (DMA stays ahead)
        DR = mybir.MatmulPerfMode.DoubleRow
        for j in range(CT):
            for t in range(RT):
                ps = psums.tile([128, 512], dt.float32, tag="ps")
                nc.tensor.matmul(
                    ps,
                    seedL_sb[:, :, t * 128 : (t + 1) * 128],
                    seedR_sb[:, :, j * 512 : (j + 1) * 512],
                    start=True,
                    stop=False,
                    perf_mode=DR,
                )
                if j == 2 * t or j == 2 * t + 1:
                    half = j - 2 * t
                    nc.tensor.matmul(
                        ps,
                        negci_sb,
                        bcast2(dsel_sb[:, half * 512 : (half + 1) * 512]),
                        start=False,
                        stop=False,
                        perf_mode=DR,
                    )
                for kh in range(KT // 2):
                    nc.tensor.matmul(
                        ps,
                        ftTc_sb[:, 2 * kh : 2 * kh + 2, t * 128 : (t + 1) * 128],
                        ftT_sb[:, 2 * kh : 2 * kh + 2, j * 512 : (j + 1) * 512],
                        start=False,
                        stop=(kh == KT // 2 - 1),
                        perf_mode=DR,
                    )
                # sgn = 2*eq-1 directly from the one-hot DR matmul
                eq = eqps.tile([128, 512], dt.float32, tag="eq")
                nc.tensor.matmul(
                    eq,
                    ohTc_sb[:, :, t * 128 : (t + 1) * 128],
                    bcast2(ohT_sb[:, j * 512 : (j + 1) * 512]),
                    start=True,
                    stop=True,
                    perf_mode=DR,
                )
                # psum = gram + (512-sq_i/2) + (512-sq_j/2) - C*diag
                # -> -2*psum + 2048 = d2 + 2C*diag > 0
                idx = t * CT + j
                dist = work.tile([128, 512], dt.float32, tag="dist")
                nc.scalar.activation(
                    dist, ps, Act.Sqrt, scale=-2.0, bias=bias2048[:]
                )
                scr = work.tile([128, 512], dt.float32, tag="scr")
                nc.vector.scalar_tensor_tensor(
                    scr,
                    dist,
                    1.0,
                    eq,
                    Alu.mult,
                    Alu.mult,
                    accum_out=acc_sb[:, idx : idx + 1],
                )

        nc.sync.dma_start(rowacc, acc_sb)

    nc.finalize()
    return nc


def _get_bass():
    if "nc" not in _BASS_CACHE:
        _BASS_CACHE["nc"] = _build_bass()
    return _BASS_CACHE["nc"]


def kernel(features, y):
    global LAST_RESULTS
    from concourse.bass_utils import run_bass_kernel_spmd

    f32 = np.float32
    bf16 = ml_dtypes.bfloat16
    feats = np.asarray(features, dtype=f32)
    yv = np.asarray(y).reshape(-1)

    fb = feats.astype(bf16)
    sq32 = (feats.astype(np.float64) ** 2).sum(axis=1)
    rnorm = 1.0 / np.maximum(np.sqrt(sq32), COS_EPS)
    if USE_FP8:
        # TRN fp8e4 saturates at +-240; randn data never reaches it but clip
        # defensively so out-of-range values can't become inf
        fq = np.clip(feats, -240.0, 240.0).astype(ml_dtypes.float8_e4m3)
        ftT_np = np.ascontiguousarray(fq.T)                  # [D, N] fp8
        # fp32 row norms make the fp8 quantization error on d2 unbiased:
        # E[sq32_i + sq32_j - 2*gram_q_ij] = d2_ref
        seedsq = sq32
        sq_quant = (fq.astype(np.float64) ** 2).sum(axis=1)
    else:
        fb32 = fb.astype(f32)
        ftT_np = np.ascontiguousarray(fb.T)                  # [D, N] bf16
        seedsq = (fb32.astype(np.float64) ** 2).sum(axis=1)  # matches gram diag
        sq_quant = seedsq
    # seeds centered at 512, split hi+lo in fp8 (residual error ~0.25)
    fp8 = ml_dtypes.float8_e4m3
    cfull = 512.0 - 0.5 * seedsq
    c_hi = cfull.astype(f32).astype(fp8)
    c_lo = (cfull - c_hi.astype(np.float64)).astype(f32).astype(fp8)
    cvals = c_hi.astype(np.float64) + c_lo.astype(np.float64)
    # device computes dist_ii = sqrt(-2*sq_quant_i - 4*c_i + 2048 + 2C); subtract it
    diagcorr = float(
        np.sqrt(
            np.maximum(-2.0 * sq_quant - 4.0 * cvals + 2048.0 + 2.0 * DIAG_C, 0.0)
        ).sum()
    )
    ones_n = np.ones(N, fp8)
    seedR_np = np.ascontiguousarray(
        np.stack([np.stack([ones_n, c_hi]), np.stack([ones_n, c_lo])])
    )  # [2, 2, N]
    negci_np = np.zeros((128, 2, 128), fp8)
    negci_np[np.arange(128), 0, np.arange(128)] = fp8(-8.0)
    dsel_all = np.zeros((NCORES, 128, 1024), fp8)
    for c in range(NCORES):
        dsel_all[c, np.arange(128), c + 8 * np.arange(128)] = fp8(64.0)
    fcast = ml_dtypes.float8_e4m3 if USE_FP8 else bf16
    fchunk_src = fq if USE_FP8 else fb
    lbl_f32 = yv.astype(f32)
    onehot = (yv[:, None] == np.arange(NCLS)[None, :]).astype(f32)
    oh_ext = np.concatenate([onehot, np.ones((N, 1), f32)], axis=1).astype(fcast)
    # sgn-DR operands: rhs rows = onehot + (-ones); lhsT rows = 2*onehot + ones
    ohTdr_np = np.zeros((128, N), fp8)
    ohTdr_np[:NCLS, :] = onehot.T.astype(fp8)
    ohTdr_np[NCLS, :] = fp8(-1.0)
    ohTdrc_all = np.zeros((NCORES, 128, 2, CHUNK), fp8)
    for c in range(NCORES):
        ohTdrc_all[c, :NCLS, 0, :] = (2.0 * onehot[c::NCORES].T).astype(fp8)
        ohTdrc_all[c, NCLS, 0, :] = fp8(1.0)

    in_maps = []
    for c in range(NCORES):
        sl = slice(c, None, NCORES)  # interleaved rows: core c gets c::8
        in_maps.append(
            {
                "ftT": ftT_np,
                "ftTc": np.ascontiguousarray(ftT_np[:, sl]),
                "fc": np.ascontiguousarray(fchunk_src[sl]),
                "seeddrL": np.ascontiguousarray(
                    np.stack(
                        [
                            np.stack([c_hi[sl], np.ones(CHUNK, fp8)]),
                            np.stack([c_lo[sl], np.ones(CHUNK, fp8)]),
                        ]
                    )
                ),
                "seeddrR": seedR_np,
                "ohTdr": ohTdr_np,
                "ohTdrc": np.ascontiguousarray(ohTdrc_all[c]),
                "rnc": np.ascontiguousarray(
                    rnorm[sl].astype(f32).reshape(RT, 128).T
                ),
                "oh": np.ascontiguousarray(oh_ext[sl]),
                "negci": np.ascontiguousarray(negci_np),
                "dsel": np.ascontiguousarray(dsel_all[c]),
            }
        )

    global _LAST_IN_MAPS
    _LAST_IN_MAPS = in_maps
    nc = _get_bass()
    res = None
    for attempt in range(3):
        try:
            res = run_bass_kernel_spmd(nc, in_maps, core_ids=list(range(NCORES)))
            break
        except Exception:
            # the axon-tunneled device occasionally drops with
            # NRT_EXEC_UNIT_UNRECOVERABLE and recovers on retry
            if attempt == 2:
                raise
            import time

            time.sleep(10)
    LAST_RESULTS = res

    q = 0.0
    for r in res.results:
        q += r["rowacc"].astype(np.float64).sum()
    # rowacc = sum(sgn*dist) with the shifted diagonal contributing +dist_ii
    A = q - diagcorr

    SC = np.zeros((NCLS + 1, D), np.float64)
    for r in res.results:
        SC += r["scpart"].astype(np.float64)
    B = 2.0 * np.sum(SC[:NCLS] ** 2) - np.sum(SC[NCLS] ** 2)

    return np.asarray(A - B, dtype=f32)
